# revision 14
# baseline (speedup 1.0000x reference)
"""Deformable conv (nn_DeformConv_31267361915085) Trainium2 Bass kernel.

Sharding: data-parallel over (batch, H-half): core n handles batch n//2,
output rows [28*(n%2), 28*(n%2)+28). Weights replicated. SPMD: one program;
per-core input slabs are pre-shifted on host so the program is core-agnostic.

Per-core pipeline (on device):
  1. offset conv: 9 taps x 2 c-chunks of bf16 matmuls, PSUM-accumulated
  2. PE-transpose offsets to pixel-on-partition layout; fp32 coordinate and
     bilinear-weight math on DVE (floor via int cast + compare fixup)
  3. dma_gather (prepare_only + trigger, so the Pool engine is only held for
     descriptor generation) of 2x2 "quad" corner vectors (bf16, 2KB elements)
     from a zero-padded channels-last quad table in DRAM; quad rows are packed
     [y0x0, y1x0, y0x1, y1x1] so both lerp stages see contiguous halves
  4. bilinear lerp as tensor_scalar (4x DVE mode) + tensor_tensor (2x) ops
     with per-partition (= per-pixel) weights; scalar_tensor_tensor is avoided
     (it has no fast DVE uops and runs 1x)
  5. PE-transpose patches to [ck, pixel] layout, 18-chunk bf16 matmul with
     the main conv weights in 512-column groups that start as soon as their
     four blocks are lerped, PSUM accumulate, DMA out per group.
"""

import sys

if "/opt/trn_rl_repo" not in sys.path:
    sys.path.insert(0, "/opt/trn_rl_repo")

import contextlib

import numpy as np
import ml_dtypes

import concourse.bass as bass
import concourse.tile as tile
from concourse import bacc, mybir
from concourse.bass_utils import run_bass_kernel_spmd
from concourse.masks import make_identity

F32 = mybir.dt.float32
BF16 = mybir.dt.bfloat16
I16 = mybir.dt.int16
I32 = mybir.dt.int32
AL = mybir.AluOpType

# problem dims
B, CIN, H, W = 4, 256, 56, 56
COUT = 256
KK = 9
MARG = 8                # gather pad margin (covers |offset| <= ~6)
HQ = WQ = H + 2 * MARG  # 72: quad-table grid
NQ = HQ * WQ            # 5184 quad rows
NROWS = 28              # output rows per core
NPIX = NROWS * W        # 1568
BLK = 112               # pixels per block (2 output rows)
NBLK = NPIX // BLK      # 14
SLOT = 128              # gather slots per (tap, block): 112 real + 16 pad
NIDX = KK * SLOT        # 1152 gather indices per block
NSLOT = NBLK * SLOT     # 1792 slot-columns
# main-matmul column groups: 4 blocks = 512 slots each (last group 256)
GROUPS = [(0, 512), (512, 512), (1024, 512), (1536, 256)]

_CACHE = {}


def _ap(base, offset_elems, dims):
    """AP with explicit free dims on top of a tile's base AP."""
    return bass.AP(
        tensor=base.tensor, offset=base.offset + offset_elems, ap=[base.ap[0]] + dims
    )


def build_nc():
    # 2048-descriptor SWDGE ring so two block gathers (1152 descriptors each)
    # fit in flight: the next gather's descriptor generation overlaps the
    # previous gather's transfer instead of stalling on ring space
    nc = bacc.Bacc(
        None,
        target_bir_lowering=False,
        dynamic_dma_scratch_size=32768,
    )

    xcf_d = nc.dram_tensor("xcf", [128, 2, 30 * 58], BF16, kind="ExternalInput")
    xq_d = nc.dram_tensor("xq", [NQ, 1024], BF16, kind="ExternalInput")
    woff_d = nc.dram_tensor("woff", [128, 2, KK, 18], BF16, kind="ExternalInput")
    boff_d = nc.dram_tensor("boff", [18, 1], F32, kind="ExternalInput")
    wm_d = nc.dram_tensor("wm", [128, KK, 2, 2, 128], BF16, kind="ExternalInput")
    out_d = nc.dram_tensor("out", [128, 2, NSLOT], F32, kind="ExternalOutput")

    with tile.TileContext(nc) as tc, contextlib.ExitStack() as ctx:
        singles = ctx.enter_context(tc.tile_pool(name="singles", bufs=1))
        coords = ctx.enter_context(tc.tile_pool(name="coords", bufs=1))
        dramp = ctx.enter_context(tc.tile_pool(name="dramp", bufs=1, space="DRAM"))

        # ---- load constants / weights / activations ----
        xcf = singles.tile([128, 2, 30 * 58], BF16)
        nc.sync.dma_start(out=xcf[:, :, :], in_=xcf_d[:, :, :])
        woff = singles.tile([128, 2, KK, 18], BF16)
        nc.sync.dma_start(out=woff[:, :, :, :], in_=woff_d[:, :, :, :])
        boff = singles.tile([18, 1], F32)
        nc.sync.dma_start(out=boff[:, :], in_=boff_d[:, :])
        wm = singles.tile([128, KK, 2, 2, 128], BF16)
        nc.sync.dma_start(out=wm[:, :, :, :, :], in_=wm_d[:, :, :, :, :])

        ident_f = singles.tile([128, 128], F32)
        make_identity(nc, ident_f[:, :])
        ident_b = singles.tile([128, 128], BF16)
        nc.vector.tensor_copy(out=ident_b[:, :], in_=ident_f[:, :])

        # warmup gather: the first DMAGatherAnt on a core pays a ~12us
        # one-time cost (Q7 library/ring init); hide it under the offset conv
        widx = singles.tile([128, 8], I16)
        nc.gpsimd.memset(widx[:, :], 0)
        wg = singles.tile([128, 1, 1024], BF16)
        nc.gpsimd.dma_gather(
            out_ap=wg[:, :, :],
            in_ap=xq_d[:, :],
            idxs_ap=widx[:, :],
            num_idxs=128,
            num_idxs_reg=128,
            elem_size=1024,
            single_packet=False,
        )

        # iota-derived planes (core-independent)
        it_i = coords.tile([128, 1], I32)
        nc.gpsimd.iota(it_i[:, :], pattern=[[0, 1]], base=0, channel_multiplier=1)
        p_f = coords.tile([128, 1], F32)
        nc.vector.tensor_copy(out=p_f[:, :], in_=it_i[:, :])
        pge = coords.tile([128, 1], F32)  # 1.0 if partition >= 56
        nc.vector.tensor_scalar(
            out=pge[:, :], in0=p_f[:, :], scalar1=56.0, scalar2=None, op0=AL.is_ge
        )
        jx = coords.tile([128, 1], F32)  # j = p - 56*(p>=56)
        nc.vector.scalar_tensor_tensor(
            out=jx[:, :], in0=pge[:, :], scalar=-56.0, in1=p_f[:, :],
            op0=AL.mult, op1=AL.add,
        )
        bb2_i = coords.tile([128, NBLK], I32)
        nc.gpsimd.iota(bb2_i[:, :], pattern=[[2, NBLK]], base=0, channel_multiplier=0)
        iy2 = coords.tile([128, NBLK], F32)  # block-local row: 2*bb + (p>=56)
        nc.vector.tensor_copy(out=iy2[:, :], in_=bb2_i[:, :])
        nc.vector.tensor_tensor(
            out=iy2[:, :], in0=iy2[:, :], in1=_ap(pge[:], 0, [[0, NBLK]]), op=AL.add
        )
        kyM_i = coords.tile([128, KK], I32)
        nc.gpsimd.iota(
            kyM_i[:, :], pattern=[[1, 3], [0, 3]], base=MARG - 1, channel_multiplier=0
        )
        kyM = coords.tile([128, KK], F32)
        nc.vector.tensor_copy(out=kyM[:, :], in_=kyM_i[:, :])
        kxM_i = coords.tile([128, KK], I32)
        nc.gpsimd.iota(
            kxM_i[:, :], pattern=[[0, 3], [1, 3]], base=MARG - 1, channel_multiplier=0
        )
        kxM = coords.tile([128, KK], F32)
        nc.vector.tensor_copy(out=kxM[:, :], in_=kxM_i[:, :])

        # ---- offset conv ----
        off_sb = coords.tile([18, 4 * 392], F32)
        with tc.tile_pool(name="po", bufs=2, space="PSUM") as po:
            for ns in range(4):
                ps_o = po.tile([18, 392], F32)
                for kc in range(18):
                    k, ch = divmod(kc, 2)
                    ky, kx = divmod(k, 3)
                    rhs = _ap(
                        xcf[:, :, :],
                        ch * 1740 + (ns * 7 + ky) * 58 + kx,
                        [[58, 7], [1, 56]],
                    )
                    nc.tensor.matmul(
                        ps_o[:, :],
                        woff[:, ch, k, :],
                        rhs,
                        start=(kc == 0),
                        stop=(kc == 17),
                    )
                nc.vector.tensor_scalar(
                    out=off_sb[:, ns * 392 : (ns + 1) * 392],
                    in0=ps_o[:, :],
                    scalar1=boff[:, 0:1],
                    scalar2=None,
                    op0=AL.add,
                )

        # ---- transpose offsets to pixel-on-partition ----
        offT = coords.tile([128, NBLK, 18], F32)
        nc.vector.memset(offT[:, :, :], 0.0)
        with tc.tile_pool(name="pot", bufs=2, space="PSUM") as pot:
            for bb in range(NBLK):
                ps_t = pot.tile([112, 18], F32)
                nc.tensor.transpose(
                    ps_t[:, :],
                    off_sb[:18, bb * BLK : (bb + 1) * BLK],
                    ident_f[:18, :18],
                )
                nc.vector.tensor_copy(out=offT[:112, bb, :], in_=ps_t[:, :])

        # ---- coordinate + weight math (fp32 [128, NBLK, 9] planes) ----
        _pc = [0]

        def plane():
            _pc[0] += 1
            return coords.tile([128, NBLK, KK], F32, name=f"cplane{_pc[0]}")

        dy = _ap(offT[:], 0, [[18, NBLK], [2, KK]])
        dx = _ap(offT[:], 1, [[18, NBLK], [2, KK]])
        iy_b = _ap(iy2[:], 0, [[1, NBLK], [0, KK]])
        jx_b = _ap(jx[:], 0, [[0, NBLK], [0, KK]])
        kyM_b = _ap(kyM[:], 0, [[0, NBLK], [1, KK]])
        kxM_b = _ap(kxM[:], 0, [[0, NBLK], [1, KK]])

        pym = coords.tile([128, NBLK, KK], F32)
        pxm = coords.tile([128, NBLK, KK], F32)
        nc.vector.tensor_tensor(out=pym[:, :, :], in0=dy, in1=iy_b, op=AL.add)
        nc.vector.tensor_tensor(out=pym[:, :, :], in0=pym[:, :, :], in1=kyM_b, op=AL.add)
        nc.vector.tensor_tensor(out=pxm[:, :, :], in0=dx, in1=jx_b, op=AL.add)
        nc.vector.tensor_tensor(out=pxm[:, :, :], in0=pxm[:, :, :], in1=kxM_b, op=AL.add)

        def floor_of(src):
            ci = coords.tile([128, NBLK, KK], I32, name=f"ci{_pc[0]}")
            nc.vector.tensor_copy(out=ci[:, :, :], in_=src[:, :, :])
            cf = plane()
            nc.vector.tensor_copy(out=cf[:, :, :], in_=ci[:, :, :])
            gt = plane()
            nc.vector.tensor_tensor(
                out=gt[:, :, :], in0=cf[:, :, :], in1=src[:, :, :], op=AL.is_gt
            )
            nc.vector.tensor_tensor(
                out=cf[:, :, :], in0=cf[:, :, :], in1=gt[:, :, :], op=AL.subtract
            )
            return cf

        y0 = floor_of(pym)
        x0 = floor_of(pxm)
        ty = coords.tile([128, NBLK, KK], F32)
        tx = coords.tile([128, NBLK, KK], F32)
        nc.vector.tensor_tensor(
            out=ty[:, :, :], in0=pym[:, :, :], in1=y0[:, :, :], op=AL.subtract
        )
        nc.vector.tensor_tensor(
            out=tx[:, :, :], in0=pxm[:, :, :], in1=x0[:, :, :], op=AL.subtract
        )
        tyc = coords.tile([128, NBLK, KK], F32)  # 1 - ty
        txc = coords.tile([128, NBLK, KK], F32)  # 1 - tx
        nc.vector.tensor_scalar(
            out=tyc[:, :, :], in0=ty[:, :, :], scalar1=-1.0, scalar2=1.0,
            op0=AL.mult, op1=AL.add,
        )
        nc.vector.tensor_scalar(
            out=txc[:, :, :], in0=tx[:, :, :], scalar1=-1.0, scalar2=1.0,
            op0=AL.mult, op1=AL.add,
        )

        # clamp into quad table (clamped region is zero-padded -> exact)
        y0c, x0c = plane(), plane()
        nc.vector.tensor_scalar(
            out=y0c[:, :, :], in0=y0[:, :, :], scalar1=0.0, scalar2=float(HQ - 1),
            op0=AL.max, op1=AL.min,
        )
        nc.vector.tensor_scalar(
            out=x0c[:, :, :], in0=x0[:, :, :], scalar1=0.0, scalar2=float(WQ - 1),
            op0=AL.max, op1=AL.min,
        )
        idxf = plane()
        nc.vector.scalar_tensor_tensor(
            out=idxf[:, :, :], in0=y0c[:, :, :], scalar=float(WQ), in1=x0c[:, :, :],
            op0=AL.mult, op1=AL.add,
        )

        # ---- fold indices into SWDGE wrapped layout (via DRAM round trip) ----
        # idxw[q + 16r, bb, k*8 + t] = idx(tap k, pixel 16t + q) of block bb.
        # idxT16 columns are permuted to (q, t) order (col q*8+t = pixel
        # 16t+q), so the DRAM wrap read has 16-byte contiguous runs.
        idxT16 = coords.tile([126, 128], I16)
        with tc.tile_pool(name="pidx", bufs=1, space="PSUM") as pidx:
            ps_i = pidx.tile([126, 128], F32)
            nc.tensor.transpose(
                ps_i[:, :], _ap(idxf[:, :, :], 0, [[1, 126]]), ident_f[:, :]
            )
            nc.vector.tensor_copy(
                out=idxT16[:, :], in_=_ap(ps_i[:, :], 0, [[1, 16], [16, 8]])
            )
        idxd = dramp.tile([126, 128], I16)
        nc.sync.dma_start(out=idxd[:, :], in_=idxT16[:, :])
        idxw = coords.tile([128, NBLK, 72], I16)
        iw = idxw[:, :, :]
        ppw = iw.ap[0][0]
        idb = idxd[:, :]
        # wrap read into partitions 0..15: dst col k*8+t <- dram row (bb*9+k),
        # byte col q*8+t
        dst0 = bass.AP(
            tensor=iw.tensor,
            offset=iw.offset,
            ap=[[ppw, 16], [72, NBLK], [8, KK], [1, 8]],
        )
        src0 = bass.AP(
            tensor=idb.tensor,
            offset=idb.offset,
            ap=[[8, 16], [128 * KK, NBLK], [128, KK], [1, 8]],
        )
        # SWDGE (Q7) generates these 2016 16-byte descriptors in ~1.7us;
        # HWDGE would take ~20us at its per-descriptor rate
        nc.gpsimd.dma_start(out=dst0, in_=src0)
        # replicate to partition groups 1..7 (SBUF->SBUF, 2KB runs)
        rep = NBLK * 72
        for r in range(1, 8):
            src = bass.AP(tensor=iw.tensor, offset=iw.offset, ap=[[ppw, 16], [1, rep]])
            dst = bass.AP(
                tensor=iw.tensor,
                offset=iw.offset + 16 * r * ppw,
                ap=[[ppw, 16], [1, rep]],
            )
            eng = nc.sync if r % 2 else nc.scalar
            eng.dma_start(out=dst, in_=src)

        # ---- gather + lerp + transpose per block; main matmul per group ----
        rhs_buf = singles.tile([128, KK, 2, NSLOT], BF16)
        out_sb = singles.tile([128, 2, NSLOT], F32)
        with (
            tc.tile_pool(name="gp", bufs=3) as gp,
            tc.tile_pool(name="pp", bufs=3) as pp,
            tc.tile_pool(name="ptb", bufs=2, space="PSUM") as ptb,
            tc.tile_pool(name="pm", bufs=2, space="PSUM") as pm,
        ):
            NDVE = 2  # taps whose y-lerp stays on DVE (load balance vs ACT)
            for bb in range(NBLK):
                g = gp.tile([128, KK, 1024], BF16)
                # two sub-gathers per block (taps 0-4, taps 5-8): smaller
                # descriptor batches duck SWDGE ring-space stalls and let the
                # first taps' lerp start earlier
                nc.gpsimd.dma_gather(
                    out_ap=g[:, 0:5, :],
                    in_ap=xq_d[:, :],
                    idxs_ap=idxw[:, bb, 0:40],
                    num_idxs=5 * SLOT,
                    num_idxs_reg=5 * SLOT,
                    elem_size=1024,
                    single_packet=False,
                )
                nc.gpsimd.dma_gather(
                    out_ap=g[:, 5:KK, :],
                    in_ap=xq_d[:, :],
                    idxs_ap=_ap(idxw[:, :, :], bb * 72 + 40, [[1, 32]]),
                    num_idxs=4 * SLOT,
                    num_idxs_reg=4 * SLOT,
                    elem_size=1024,
                    single_packet=False,
                )
                ps_b = ptb.tile([128, KK, 2, 128], BF16)
                for k in range(KK):
                    gk = g[:, k, 0:1024]
                    # quad row layout: [y0x0, y1x0, y0x1, y1x1] * 256ch each.
                    # x-lerp on DVE: TT-sub (2x) + STT mult-add (1x; STT has no
                    # fast uops but 2 ops beat any 3-op formulation on HW).
                    hx = pp.tile([128, 512], BF16, tag="hx", name="hx")
                    nc.vector.tensor_tensor(
                        out=hx[:, :], in0=gk[:, 512:1024], in1=gk[:, 0:512],
                        op=AL.subtract,
                    )
                    nc.vector.scalar_tensor_tensor(
                        out=hx[:, :], in0=hx[:, :], scalar=tx[:, bb, k : k + 1],
                        in1=gk[:, 0:512], op0=AL.mult, op1=AL.add,
                    )
                    # hx = [y0 x-lerped (256) | y1 x-lerped (256)]
                    # y-lerp split between DVE (sub+STT) and the Scalar engine
                    # (two per-pixel scalings + DVE add) to balance both
                    p0 = pp.tile([128, 256], BF16, tag="p0", name="p0")
                    if k < NDVE:
                        nc.vector.tensor_tensor(
                            out=p0[:, :], in0=hx[:, 256:512], in1=hx[:, 0:256],
                            op=AL.subtract,
                        )
                        nc.vector.scalar_tensor_tensor(
                            out=p0[:, :], in0=p0[:, :],
                            scalar=ty[:, bb, k : k + 1], in1=hx[:, 0:256],
                            op0=AL.mult, op1=AL.add,
                        )
                    else:
                        p1 = pp.tile([128, 256], BF16, tag="p1", name="p1")
                        nc.scalar.activation(
                            out=p0[:, :], in_=hx[:, 0:256],
                            func=mybir.ActivationFunctionType.Copy,
                            scale=tyc[:, bb, k : k + 1],
                        )
                        nc.scalar.activation(
                            out=p1[:, :], in_=hx[:, 256:512],
                            func=mybir.ActivationFunctionType.Copy,
                            scale=ty[:, bb, k : k + 1],
                        )
                        nc.vector.tensor_tensor(
                            out=p0[:, :], in0=p0[:, :], in1=p1[:, :], op=AL.add
                        )
                    for ch in range(2):
                        nc.tensor.transpose(
                            ps_b[:, k, ch, :],
                            p0[:, ch * 128 : (ch + 1) * 128],
                            ident_b[:, :],
                        )
                nc.scalar.copy(
                    out=rhs_buf[:, :, :, bb * SLOT : (bb + 1) * SLOT],
                    in_=ps_b[:, :, :, :],
                )

            # main conv matmul per 512-col group; each group's matmuls only
            # depend on its own four blocks' rhs columns, so they overlap
            # the remaining blocks' gathers/lerps
            for c0, wdt in GROUPS:
                for ot in range(2):
                    ps = pm.tile([128, 512], F32, tag="pm", name="pmtile")
                    for kc in range(18):
                        k, ch = divmod(kc, 2)
                        nc.tensor.matmul(
                            ps[:, 0:wdt],
                            wm[:, k, ch, ot, :],
                            rhs_buf[:, k, ch, c0 : c0 + wdt],
                            start=(kc == 0),
                            stop=(kc == 17),
                        )
                    nc.scalar.copy(
                        out=out_sb[:, ot, c0 : c0 + wdt], in_=ps[:, 0:wdt]
                    )
                nc.sync.dma_start(
                    out=out_d[:, :, c0 : c0 + wdt], in_=out_sb[:, :, c0 : c0 + wdt]
                )

    nc.compile()
    return nc


def prep_inputs(x, w_off, b_off, w):
    """Host-side slab/layout prep. Returns list of 8 per-core input dicts."""
    x = np.asarray(x, dtype=np.float32)
    w_off = np.asarray(w_off, dtype=np.float32)
    b_off = np.asarray(b_off, dtype=np.float32)
    w = np.asarray(w, dtype=np.float32)

    woff_arr = np.ascontiguousarray(
        w_off.reshape(18, 2, 128, KK).transpose(2, 1, 3, 0)
    ).astype(ml_dtypes.bfloat16)  # [128 cl, 2 ch, 9 k, 18 o]
    boff_arr = np.ascontiguousarray(b_off.reshape(18, 1))
    wm_arr = np.ascontiguousarray(
        w.reshape(2, 128, 2, 128, KK).transpose(3, 4, 2, 0, 1)
    ).astype(ml_dtypes.bfloat16)  # [128 cl, 9 k, 2 ch, 2 ot, 128 ol]

    in_maps = []
    for core in range(8):
        b, half = divmod(core, 2)
        r0 = half * NROWS
        xb = x[b]  # [256, 56, 56]

        xp58 = np.zeros((CIN, 58, 58), np.float32)
        xp58[:, 1:57, 1:57] = xb
        xcf = np.ascontiguousarray(
            xp58[:, r0 : r0 + 30, :].reshape(2, 128, 30 * 58).transpose(1, 0, 2)
        ).astype(ml_dtypes.bfloat16)

        xp = np.zeros((HQ + 1, WQ + 1, CIN), np.float32)
        ylo = max(0, r0 - MARG)
        yhi = min(H, r0 + HQ + 1 - MARG)
        xhwc = xb.transpose(1, 2, 0)
        xp[ylo - (r0 - MARG) : yhi - (r0 - MARG), MARG : MARG + W, :] = xhwc[ylo:yhi]
        # quad row layout [y0x0, y1x0, y0x1, y1x1] so each lerp stage reads
        # a contiguous 512-element half
        quad = np.stack(
            [xp[:-1, :-1], xp[1:, :-1], xp[:-1, 1:], xp[1:, 1:]], axis=2
        )  # [72, 72, 4, 256]
        xq = np.ascontiguousarray(quad.reshape(NQ, 4 * CIN)).astype(ml_dtypes.bfloat16)

        in_maps.append(
            {
                "xcf": xcf,
                "xq": xq,
                "woff": woff_arr,
                "boff": boff_arr,
                "wm": wm_arr,
            }
        )
    return in_maps


def unshard_output(results):
    """results: list of 8 per-core out arrays [128, 2, NSLOT] -> [B,COUT,H,W]."""
    out = np.zeros((B, COUT, H, W), np.float32)
    for core in range(8):
        b, half = divmod(core, 2)
        r0 = half * NROWS
        oc = results[core]  # [128 ol, 2 ot, 1792]
        oc = oc.reshape(128, 2, NBLK, SLOT)[:, :, :, :BLK]
        oc = oc.transpose(1, 0, 2, 3).reshape(COUT, NROWS, W)
        out[b, :, r0 : r0 + NROWS, :] = oc
    return out


def kernel(**inputs):
    nc = _CACHE.get("nc")
    if nc is None:
        nc = build_nc()
        _CACHE["nc"] = nc
    in_maps = prep_inputs(
        inputs["x"], inputs["w_off"], inputs["b_off"], inputs["w"]
    )
    res = run_bass_kernel_spmd(nc, in_maps, core_ids=list(range(8)))
    return unshard_output([r["out"] for r in res.results])


# revision 21
# speedup vs baseline: 1.1468x; 1.1468x over previous
"""Deformable conv (nn_DeformConv_31267361915085) Trainium2 Bass kernel.

Sharding: data-parallel over (batch, H-half): core n handles batch n//2,
output rows [28*(n%2), 28*(n%2)+28). Weights replicated. SPMD: one program;
per-core input slabs are pre-shifted on host so the program is core-agnostic.

Per-core pipeline (on device):
  1. offset conv: 9 taps x 2 c-chunks of bf16 matmuls, PSUM-accumulated
  2. PE-transpose offsets to pixel-on-partition layout; fp32 coordinate and
     bilinear-weight math on DVE (floor via int cast + compare fixup)
  3. dma_gather (prepare_only + trigger, so the Pool engine is only held for
     descriptor generation) of 2x2 "quad" corner vectors (bf16, 2KB elements)
     from a zero-padded channels-last quad table in DRAM; quad rows are packed
     [y0x0, y1x0, y0x1, y1x1] so both lerp stages see contiguous halves
  4. bilinear lerp as tensor_scalar (4x DVE mode) + tensor_tensor (2x) ops
     with per-partition (= per-pixel) weights; scalar_tensor_tensor is avoided
     (it has no fast DVE uops and runs 1x)
  5. PE-transpose patches to [ck, pixel] layout, 18-chunk bf16 matmul with
     the main conv weights in 512-column groups that start as soon as their
     four blocks are lerped, PSUM accumulate, DMA out per group.
"""

import sys

if "/opt/trn_rl_repo" not in sys.path:
    sys.path.insert(0, "/opt/trn_rl_repo")

import contextlib

import numpy as np
import ml_dtypes

import concourse.bass as bass
import concourse.tile as tile
from concourse import bacc, mybir
from concourse.bass_utils import run_bass_kernel_spmd
from concourse.masks import make_identity

F32 = mybir.dt.float32
BF16 = mybir.dt.bfloat16
I16 = mybir.dt.int16
I32 = mybir.dt.int32
AL = mybir.AluOpType

# problem dims
B, CIN, H, W = 4, 256, 56, 56
COUT = 256
KK = 9
MARG = 8                # gather pad margin (covers |offset| <= ~6)
HQ = WQ = H + 2 * MARG  # 72: quad-table grid
NQ = HQ * WQ            # 5184 quad rows
NROWS = 28              # output rows per core
NPIX = NROWS * W        # 1568
BLK = 112               # pixels per block (2 output rows)
NBLK = NPIX // BLK      # 14
SLOT = 128              # gather slots per (tap, block): 112 real + 16 pad
NIDX = KK * SLOT        # 1152 gather indices per block
NSLOT = NBLK * SLOT     # 1792 slot-columns
# main-matmul column groups: 4 blocks = 512 slots each (last group 256)
GROUPS = [(0, 512), (512, 512), (1024, 512), (1536, 256)]

_CACHE = {}


def _ap(base, offset_elems, dims):
    """AP with explicit free dims on top of a tile's base AP."""
    return bass.AP(
        tensor=base.tensor, offset=base.offset + offset_elems, ap=[base.ap[0]] + dims
    )


def build_nc():
    # 2048-descriptor SWDGE ring so two block gathers (1152 descriptors each)
    # fit in flight: the next gather's descriptor generation overlaps the
    # previous gather's transfer instead of stalling on ring space
    nc = bacc.Bacc(
        None,
        target_bir_lowering=False,
        dynamic_dma_scratch_size=32768,
    )

    xcf_d = nc.dram_tensor("xcf", [128, 2, 30 * 58], BF16, kind="ExternalInput")
    xq_d = nc.dram_tensor("xq", [NQ, 1024], BF16, kind="ExternalInput")
    woff_d = nc.dram_tensor("woff", [128, 2, KK, 18], BF16, kind="ExternalInput")
    boff_d = nc.dram_tensor("boff", [18, 1], F32, kind="ExternalInput")
    wm_d = nc.dram_tensor("wm", [128, KK, 2, 2, 128], BF16, kind="ExternalInput")
    # host-precomputed planes: iy2 [NBLK], jx [1], kyM [KK], kxM [KK] f32 and
    # identities. Keeping iota/identity off the Pool engine means its Q7
    # library is never swapped away from the dma_gather overlay (a swap costs
    # ~12us before the next gather).
    cf32_d = nc.dram_tensor("cf32", [128, NBLK + 1 + 2 * KK], F32, kind="ExternalInput")
    identf_d = nc.dram_tensor("identf", [128, 128], F32, kind="ExternalInput")
    identb_d = nc.dram_tensor("identb", [128, 128], BF16, kind="ExternalInput")
    out_d = nc.dram_tensor("out", [128, 2, NSLOT], F32, kind="ExternalOutput")

    with tile.TileContext(nc) as tc, contextlib.ExitStack() as ctx:
        singles = ctx.enter_context(tc.tile_pool(name="singles", bufs=1))
        coords = ctx.enter_context(tc.tile_pool(name="coords", bufs=1))
        dramp = ctx.enter_context(tc.tile_pool(name="dramp", bufs=1, space="DRAM"))

        # ---- load constants / weights / activations ----
        xcf = singles.tile([128, 2, 30 * 58], BF16)
        nc.sync.dma_start(out=xcf[:, :, :], in_=xcf_d[:, :, :])
        woff = singles.tile([128, 2, KK, 18], BF16)
        nc.sync.dma_start(out=woff[:, :, :, :], in_=woff_d[:, :, :, :])
        boff = singles.tile([18, 1], F32)
        nc.sync.dma_start(out=boff[:, :], in_=boff_d[:, :])
        wm = singles.tile([128, KK, 2, 2, 128], BF16)
        nc.sync.dma_start(out=wm[:, :, :, :, :], in_=wm_d[:, :, :, :, :])

        cf32 = singles.tile([128, NBLK + 1 + 2 * KK], F32)
        nc.sync.dma_start(out=cf32[:, :], in_=cf32_d[:, :])
        ident_f = singles.tile([128, 128], F32)
        nc.scalar.dma_start(out=ident_f[:, :], in_=identf_d[:, :])
        ident_b = singles.tile([128, 128], BF16)
        nc.scalar.dma_start(out=ident_b[:, :], in_=identb_d[:, :])
        iy2 = cf32[:, 0:NBLK]
        jx = cf32[:, NBLK : NBLK + 1]
        kyM = cf32[:, NBLK + 1 : NBLK + 1 + KK]
        kxM = cf32[:, NBLK + 1 + KK : NBLK + 1 + 2 * KK]

        # warmup gather: the first DMAGatherAnt on a core pays a ~12us
        # one-time cost (Q7 overlay load); hide it under the offset conv
        widx = singles.tile([128, 8], I16)
        nc.vector.memset(widx[:, :], 0)
        wg = singles.tile([128, 1, 1024], BF16)
        nc.gpsimd.dma_gather(
            out_ap=wg[:, :, :],
            in_ap=xq_d[:, :],
            idxs_ap=widx[:, :],
            num_idxs=128,
            num_idxs_reg=128,
            elem_size=1024,
            single_packet=False,
        )

        # ---- offset conv ----
        off_sb = coords.tile([18, 4 * 392], F32)
        with tc.tile_pool(name="po", bufs=2, space="PSUM") as po:
            for ns in range(4):
                ps_o = po.tile([18, 392], F32)
                for kc in range(18):
                    k, ch = divmod(kc, 2)
                    ky, kx = divmod(k, 3)
                    rhs = _ap(
                        xcf[:, :, :],
                        ch * 1740 + (ns * 7 + ky) * 58 + kx,
                        [[58, 7], [1, 56]],
                    )
                    nc.tensor.matmul(
                        ps_o[:, :],
                        woff[:, ch, k, :],
                        rhs,
                        start=(kc == 0),
                        stop=(kc == 17),
                    )
                nc.vector.tensor_scalar(
                    out=off_sb[:, ns * 392 : (ns + 1) * 392],
                    in0=ps_o[:, :],
                    scalar1=boff[:, 0:1],
                    scalar2=None,
                    op0=AL.add,
                )

        # ---- transpose offsets to pixel-on-partition ----
        offT = coords.tile([128, NBLK, 18], F32)
        nc.vector.memset(offT[:, :, :], 0.0)
        with tc.tile_pool(name="pot", bufs=2, space="PSUM") as pot:
            for bb in range(NBLK):
                ps_t = pot.tile([112, 18], F32)
                nc.tensor.transpose(
                    ps_t[:, :],
                    off_sb[:18, bb * BLK : (bb + 1) * BLK],
                    ident_f[:18, :18],
                )
                nc.vector.tensor_copy(out=offT[:112, bb, :], in_=ps_t[:, :])

        # ---- coordinate + weight math (fp32 [128, NBLK, 9] planes) ----
        _pc = [0]

        def plane():
            _pc[0] += 1
            return coords.tile([128, NBLK, KK], F32, name=f"cplane{_pc[0]}")

        dy = _ap(offT[:], 0, [[18, NBLK], [2, KK]])
        dx = _ap(offT[:], 1, [[18, NBLK], [2, KK]])
        iy_b = _ap(iy2, 0, [[1, NBLK], [0, KK]])
        jx_b = _ap(jx, 0, [[0, NBLK], [0, KK]])
        kyM_b = _ap(kyM, 0, [[0, NBLK], [1, KK]])
        kxM_b = _ap(kxM, 0, [[0, NBLK], [1, KK]])

        pym = coords.tile([128, NBLK, KK], F32)
        pxm = coords.tile([128, NBLK, KK], F32)
        nc.vector.tensor_tensor(out=pym[:, :, :], in0=dy, in1=iy_b, op=AL.add)
        nc.vector.tensor_tensor(out=pym[:, :, :], in0=pym[:, :, :], in1=kyM_b, op=AL.add)
        nc.vector.tensor_tensor(out=pxm[:, :, :], in0=dx, in1=jx_b, op=AL.add)
        nc.vector.tensor_tensor(out=pxm[:, :, :], in0=pxm[:, :, :], in1=kxM_b, op=AL.add)

        def floor_of(src):
            ci = coords.tile([128, NBLK, KK], I32, name=f"ci{_pc[0]}")
            nc.vector.tensor_copy(out=ci[:, :, :], in_=src[:, :, :])
            cf = plane()
            nc.vector.tensor_copy(out=cf[:, :, :], in_=ci[:, :, :])
            gt = plane()
            nc.vector.tensor_tensor(
                out=gt[:, :, :], in0=cf[:, :, :], in1=src[:, :, :], op=AL.is_gt
            )
            nc.vector.tensor_tensor(
                out=cf[:, :, :], in0=cf[:, :, :], in1=gt[:, :, :], op=AL.subtract
            )
            return cf

        y0 = floor_of(pym)
        x0 = floor_of(pxm)
        ty = coords.tile([128, NBLK, KK], F32)
        tx = coords.tile([128, NBLK, KK], F32)
        nc.vector.tensor_tensor(
            out=ty[:, :, :], in0=pym[:, :, :], in1=y0[:, :, :], op=AL.subtract
        )
        nc.vector.tensor_tensor(
            out=tx[:, :, :], in0=pxm[:, :, :], in1=x0[:, :, :], op=AL.subtract
        )
        tyc = coords.tile([128, NBLK, KK], F32)  # 1 - ty
        txc = coords.tile([128, NBLK, KK], F32)  # 1 - tx
        nc.vector.tensor_scalar(
            out=tyc[:, :, :], in0=ty[:, :, :], scalar1=-1.0, scalar2=1.0,
            op0=AL.mult, op1=AL.add,
        )
        nc.vector.tensor_scalar(
            out=txc[:, :, :], in0=tx[:, :, :], scalar1=-1.0, scalar2=1.0,
            op0=AL.mult, op1=AL.add,
        )

        # clamp into quad table (clamped region is zero-padded -> exact)
        y0c, x0c = plane(), plane()
        nc.vector.tensor_scalar(
            out=y0c[:, :, :], in0=y0[:, :, :], scalar1=0.0, scalar2=float(HQ - 1),
            op0=AL.max, op1=AL.min,
        )
        nc.vector.tensor_scalar(
            out=x0c[:, :, :], in0=x0[:, :, :], scalar1=0.0, scalar2=float(WQ - 1),
            op0=AL.max, op1=AL.min,
        )
        idxf = plane()
        nc.vector.scalar_tensor_tensor(
            out=idxf[:, :, :], in0=y0c[:, :, :], scalar=float(WQ), in1=x0c[:, :, :],
            op0=AL.mult, op1=AL.add,
        )

        # ---- fold indices into SWDGE wrapped layout (via DRAM round trip) ----
        # idxw[q + 16r, bb, k*8 + t] = idx(tap k, pixel 16t + q) of block bb.
        # idxT16 columns are permuted to (q, t) order (col q*8+t = pixel
        # 16t+q), so the DRAM wrap read has 16-byte contiguous runs.
        idxT16 = coords.tile([126, 128], I16)
        with tc.tile_pool(name="pidx", bufs=1, space="PSUM") as pidx:
            ps_i = pidx.tile([126, 128], F32)
            nc.tensor.transpose(
                ps_i[:, :], _ap(idxf[:, :, :], 0, [[1, 126]]), ident_f[:, :]
            )
            nc.vector.tensor_copy(
                out=idxT16[:, :], in_=_ap(ps_i[:, :], 0, [[1, 16], [16, 8]])
            )
        idxd = dramp.tile([126, 128], I16)
        nc.sync.dma_start(out=idxd[:, :], in_=idxT16[:, :])
        idxw = coords.tile([128, NBLK, 72], I16)
        iw = idxw[:, :, :]
        ppw = iw.ap[0][0]
        idb = idxd[:, :]
        # wrap read into partitions 0..15: dst col k*8+t <- dram row (bb*9+k),
        # byte col q*8+t. Split across both HWDGE rings (sync + scalar) so the
        # 2016 16-byte descriptors generate in parallel; Pool must stay
        # gather-only (its Q7 library would otherwise be swapped, ~12us).
        for half, eng in ((0, nc.sync), (1, nc.scalar)):
            nb = 7
            dsth = bass.AP(
                tensor=iw.tensor,
                offset=iw.offset + half * 7 * 72,
                ap=[[ppw, 16], [72, nb], [8, KK], [1, 8]],
            )
            srch = bass.AP(
                tensor=idb.tensor,
                offset=idb.offset + half * 7 * 128 * KK,
                ap=[[8, 16], [128 * KK, nb], [128, KK], [1, 8]],
            )
            eng.dma_start(out=dsth, in_=srch)
        # replicate to partition groups 1..7 (SBUF->SBUF, 2KB runs)
        rep = NBLK * 72
        for r in range(1, 8):
            src = bass.AP(tensor=iw.tensor, offset=iw.offset, ap=[[ppw, 16], [1, rep]])
            dst = bass.AP(
                tensor=iw.tensor,
                offset=iw.offset + 16 * r * ppw,
                ap=[[ppw, 16], [1, rep]],
            )
            eng = nc.sync if r % 2 else nc.scalar
            eng.dma_start(out=dst, in_=src)

        # ---- gather + lerp + transpose per block; main matmul per group ----
        rhs_buf = singles.tile([128, KK, 2, NSLOT], BF16)
        out_sb = singles.tile([128, 2, NSLOT], F32)
        with (
            tc.tile_pool(name="gp", bufs=3) as gp,
            tc.tile_pool(name="pp", bufs=3) as pp,
            tc.tile_pool(name="ptb", bufs=2, space="PSUM") as ptb,
            tc.tile_pool(name="pm", bufs=2, space="PSUM") as pm,
        ):
            NDVE = 0  # taps whose y-lerp stays on DVE (ACT waits are elastic;
            # moving real work back to DVE measured slower)
            for bb in range(NBLK):
                g = gp.tile([128, KK, 1024], BF16)
                # two sub-gathers per block (taps 0-4, taps 5-8): smaller
                # descriptor batches duck SWDGE ring-space stalls and let the
                # first taps' lerp start earlier
                nc.gpsimd.dma_gather(
                    out_ap=g[:, 0:5, :],
                    in_ap=xq_d[:, :],
                    idxs_ap=idxw[:, bb, 0:40],
                    num_idxs=5 * SLOT,
                    num_idxs_reg=5 * SLOT,
                    elem_size=1024,
                    single_packet=False,
                )
                nc.gpsimd.dma_gather(
                    out_ap=g[:, 5:KK, :],
                    in_ap=xq_d[:, :],
                    idxs_ap=_ap(idxw[:, :, :], bb * 72 + 40, [[1, 32]]),
                    num_idxs=4 * SLOT,
                    num_idxs_reg=4 * SLOT,
                    elem_size=1024,
                    single_packet=False,
                )
                ps_b = ptb.tile([128, KK, 2, 128], BF16)
                for k in range(KK):
                    gk = g[:, k, 0:1024]
                    # quad row layout: [y0x0, y1x0, y0x1, y1x1] * 256ch each.
                    # x-lerp on DVE: TT-sub (2x) + STT mult-add (1x; STT has no
                    # fast uops but 2 ops beat any 3-op formulation on HW).
                    hx = pp.tile([128, 512], BF16, tag="hx", name="hx")
                    nc.vector.tensor_tensor(
                        out=hx[:, :], in0=gk[:, 512:1024], in1=gk[:, 0:512],
                        op=AL.subtract,
                    )
                    nc.vector.scalar_tensor_tensor(
                        out=hx[:, :], in0=hx[:, :], scalar=tx[:, bb, k : k + 1],
                        in1=gk[:, 0:512], op0=AL.mult, op1=AL.add,
                    )
                    # hx = [y0 x-lerped (256) | y1 x-lerped (256)]
                    # y-lerp split between DVE (sub+STT) and the Scalar engine
                    # (two per-pixel scalings + DVE add) to balance both
                    p0 = pp.tile([128, 256], BF16, tag="p0", name="p0")
                    if k < NDVE:
                        nc.vector.tensor_tensor(
                            out=p0[:, :], in0=hx[:, 256:512], in1=hx[:, 0:256],
                            op=AL.subtract,
                        )
                        nc.vector.scalar_tensor_tensor(
                            out=p0[:, :], in0=p0[:, :],
                            scalar=ty[:, bb, k : k + 1], in1=hx[:, 0:256],
                            op0=AL.mult, op1=AL.add,
                        )
                    else:
                        p1 = pp.tile([128, 256], BF16, tag="p1", name="p1")
                        nc.scalar.activation(
                            out=p0[:, :], in_=hx[:, 0:256],
                            func=mybir.ActivationFunctionType.Copy,
                            scale=tyc[:, bb, k : k + 1],
                        )
                        nc.scalar.activation(
                            out=p1[:, :], in_=hx[:, 256:512],
                            func=mybir.ActivationFunctionType.Copy,
                            scale=ty[:, bb, k : k + 1],
                        )
                        nc.vector.tensor_tensor(
                            out=p0[:, :], in0=p0[:, :], in1=p1[:, :], op=AL.add
                        )
                    for ch in range(2):
                        nc.tensor.transpose(
                            ps_b[:, k, ch, :],
                            p0[:, ch * 128 : (ch + 1) * 128],
                            ident_b[:, :],
                        )
                nc.scalar.copy(
                    out=rhs_buf[:, :, :, bb * SLOT : (bb + 1) * SLOT],
                    in_=ps_b[:, :, :, :],
                )

            # main conv matmul per 512-col group; each group's matmuls only
            # depend on its own four blocks' rhs columns, so they overlap
            # the remaining blocks' gathers/lerps
            for c0, wdt in GROUPS:
                for ot in range(2):
                    ps = pm.tile([128, 512], F32, tag="pm", name="pmtile")
                    for kc in range(18):
                        k, ch = divmod(kc, 2)
                        nc.tensor.matmul(
                            ps[:, 0:wdt],
                            wm[:, k, ch, ot, :],
                            rhs_buf[:, k, ch, c0 : c0 + wdt],
                            start=(kc == 0),
                            stop=(kc == 17),
                        )
                    nc.scalar.copy(
                        out=out_sb[:, ot, c0 : c0 + wdt], in_=ps[:, 0:wdt]
                    )
                nc.sync.dma_start(
                    out=out_d[:, :, c0 : c0 + wdt], in_=out_sb[:, :, c0 : c0 + wdt]
                )

    nc.compile()
    return nc


def prep_inputs(x, w_off, b_off, w):
    """Host-side slab/layout prep. Returns list of 8 per-core input dicts."""
    x = np.asarray(x, dtype=np.float32)
    w_off = np.asarray(w_off, dtype=np.float32)
    b_off = np.asarray(b_off, dtype=np.float32)
    w = np.asarray(w, dtype=np.float32)

    woff_arr = np.ascontiguousarray(
        w_off.reshape(18, 2, 128, KK).transpose(2, 1, 3, 0)
    ).astype(ml_dtypes.bfloat16)  # [128 cl, 2 ch, 9 k, 18 o]
    boff_arr = np.ascontiguousarray(b_off.reshape(18, 1))
    wm_arr = np.ascontiguousarray(
        w.reshape(2, 128, 2, 128, KK).transpose(3, 4, 2, 0, 1)
    ).astype(ml_dtypes.bfloat16)  # [128 cl, 9 k, 2 ch, 2 ot, 128 ol]

    # host-precomputed coordinate planes + identities (core-independent)
    p = np.arange(128)
    half_p = (p >= 56).astype(np.float32)
    cf32_arr = np.zeros((128, NBLK + 1 + 2 * KK), np.float32)
    cf32_arr[:, 0:NBLK] = 2 * np.arange(NBLK)[None, :] + half_p[:, None]
    cf32_arr[:, NBLK] = p - 56 * half_p
    k = np.arange(KK)
    cf32_arr[:, NBLK + 1 : NBLK + 1 + KK] = (MARG - 1 + k // 3)[None, :]
    cf32_arr[:, NBLK + 1 + KK :] = (MARG - 1 + k % 3)[None, :]
    identf_arr = np.eye(128, dtype=np.float32)
    identb_arr = np.eye(128).astype(ml_dtypes.bfloat16)

    in_maps = []
    for core in range(8):
        b, half = divmod(core, 2)
        r0 = half * NROWS
        xb = x[b]  # [256, 56, 56]

        xp58 = np.zeros((CIN, 58, 58), np.float32)
        xp58[:, 1:57, 1:57] = xb
        xcf = np.ascontiguousarray(
            xp58[:, r0 : r0 + 30, :].reshape(2, 128, 30 * 58).transpose(1, 0, 2)
        ).astype(ml_dtypes.bfloat16)

        xp = np.zeros((HQ + 1, WQ + 1, CIN), np.float32)
        ylo = max(0, r0 - MARG)
        yhi = min(H, r0 + HQ + 1 - MARG)
        xhwc = xb.transpose(1, 2, 0)
        xp[ylo - (r0 - MARG) : yhi - (r0 - MARG), MARG : MARG + W, :] = xhwc[ylo:yhi]
        # quad row layout [y0x0, y1x0, y0x1, y1x1] so each lerp stage reads
        # a contiguous 512-element half
        quad = np.stack(
            [xp[:-1, :-1], xp[1:, :-1], xp[:-1, 1:], xp[1:, 1:]], axis=2
        )  # [72, 72, 4, 256]
        xq = np.ascontiguousarray(quad.reshape(NQ, 4 * CIN)).astype(ml_dtypes.bfloat16)

        in_maps.append(
            {
                "xcf": xcf,
                "xq": xq,
                "woff": woff_arr,
                "boff": boff_arr,
                "wm": wm_arr,
                "cf32": cf32_arr,
                "identf": identf_arr,
                "identb": identb_arr,
            }
        )
    return in_maps


def unshard_output(results):
    """results: list of 8 per-core out arrays [128, 2, NSLOT] -> [B,COUT,H,W]."""
    out = np.zeros((B, COUT, H, W), np.float32)
    for core in range(8):
        b, half = divmod(core, 2)
        r0 = half * NROWS
        oc = results[core]  # [128 ol, 2 ot, 1792]
        oc = oc.reshape(128, 2, NBLK, SLOT)[:, :, :, :BLK]
        oc = oc.transpose(1, 0, 2, 3).reshape(COUT, NROWS, W)
        out[b, :, r0 : r0 + NROWS, :] = oc
    return out


def kernel(**inputs):
    nc = _CACHE.get("nc")
    if nc is None:
        nc = build_nc()
        _CACHE["nc"] = nc
    in_maps = prep_inputs(
        inputs["x"], inputs["w_off"], inputs["b_off"], inputs["w"]
    )
    res = run_bass_kernel_spmd(nc, in_maps, core_ids=list(range(8)))
    return unshard_output([r["out"] for r in res.results])


# revision 22
# speedup vs baseline: 1.1865x; 1.0346x over previous
"""Deformable conv (nn_DeformConv_31267361915085) Trainium2 Bass kernel.

Sharding: data-parallel over (batch, H-half): core n handles batch n//2,
output rows [28*(n%2), 28*(n%2)+28). Weights replicated. SPMD: one program;
per-core input slabs are pre-shifted on host so the program is core-agnostic.

Per-core pipeline (on device):
  1. offset conv: 9 taps x 2 c-chunks of bf16 matmuls, PSUM-accumulated
  2. PE-transpose offsets to pixel-on-partition layout; fp32 coordinate and
     bilinear-weight math on DVE (floor via int cast + compare fixup)
  3. dma_gather (prepare_only + trigger, so the Pool engine is only held for
     descriptor generation) of 2x2 "quad" corner vectors (bf16, 2KB elements)
     from a zero-padded channels-last quad table in DRAM; quad rows are packed
     [y0x0, y1x0, y0x1, y1x1] so both lerp stages see contiguous halves
  4. bilinear lerp as tensor_scalar (4x DVE mode) + tensor_tensor (2x) ops
     with per-partition (= per-pixel) weights; scalar_tensor_tensor is avoided
     (it has no fast DVE uops and runs 1x)
  5. PE-transpose patches to [ck, pixel] layout, 18-chunk bf16 matmul with
     the main conv weights in 512-column groups that start as soon as their
     four blocks are lerped, PSUM accumulate, DMA out per group.
"""

import sys

if "/opt/trn_rl_repo" not in sys.path:
    sys.path.insert(0, "/opt/trn_rl_repo")

import contextlib

import numpy as np
import ml_dtypes

import concourse.bass as bass
import concourse.tile as tile
from concourse import bacc, mybir
from concourse.bass_utils import run_bass_kernel_spmd
from concourse.masks import make_identity

F32 = mybir.dt.float32
BF16 = mybir.dt.bfloat16
I16 = mybir.dt.int16
I32 = mybir.dt.int32
AL = mybir.AluOpType

# problem dims
B, CIN, H, W = 4, 256, 56, 56
COUT = 256
KK = 9
MARG = 8                # gather pad margin (covers |offset| <= ~6)
HQ = WQ = H + 2 * MARG  # 72: quad-table grid
NQ = HQ * WQ            # 5184 quad rows
NROWS = 28              # output rows per core
NPIX = NROWS * W        # 1568
BLK = 112               # pixels per block (2 output rows)
NBLK = NPIX // BLK      # 14
SLOT = 128              # gather slots per (tap, block): 112 real + 16 pad
NIDX = KK * SLOT        # 1152 gather indices per block
NSLOT = NBLK * SLOT     # 1792 slot-columns
# main-matmul column groups: 4 blocks = 512 slots each (last group 256)
GROUPS = [(0, 512), (512, 512), (1024, 512), (1536, 256)]

_CACHE = {}


def _ap(base, offset_elems, dims):
    """AP with explicit free dims on top of a tile's base AP."""
    return bass.AP(
        tensor=base.tensor, offset=base.offset + offset_elems, ap=[base.ap[0]] + dims
    )


def build_nc():
    # 2048-descriptor SWDGE ring so two block gathers (1152 descriptors each)
    # fit in flight: the next gather's descriptor generation overlaps the
    # previous gather's transfer instead of stalling on ring space
    nc = bacc.Bacc(
        None,
        target_bir_lowering=False,
        dynamic_dma_scratch_size=32768,
    )

    xcf_d = nc.dram_tensor("xcf", [128, 2, 30 * 58], BF16, kind="ExternalInput")
    xq_d = nc.dram_tensor("xq", [NQ, 1024], BF16, kind="ExternalInput")
    woff_d = nc.dram_tensor("woff", [128, 2, KK, 18], BF16, kind="ExternalInput")
    boff_d = nc.dram_tensor("boff", [18, 1], F32, kind="ExternalInput")
    wm_d = nc.dram_tensor("wm", [128, KK, 2, 2, 128], BF16, kind="ExternalInput")
    # host-precomputed planes: iy2 [NBLK], jx [1], kyM [KK], kxM [KK] f32 and
    # identities. Keeping iota/identity off the Pool engine means its Q7
    # library is never swapped away from the dma_gather overlay (a swap costs
    # ~12us before the next gather).
    cf32_d = nc.dram_tensor("cf32", [128, NBLK + 1 + 2 * KK], F32, kind="ExternalInput")
    identf_d = nc.dram_tensor("identf", [128, 128], F32, kind="ExternalInput")
    identb_d = nc.dram_tensor("identb", [128, 128], BF16, kind="ExternalInput")
    out_d = nc.dram_tensor("out", [128, 2, NSLOT], F32, kind="ExternalOutput")

    with tile.TileContext(nc) as tc, contextlib.ExitStack() as ctx:
        singles = ctx.enter_context(tc.tile_pool(name="singles", bufs=1))
        coords = ctx.enter_context(tc.tile_pool(name="coords", bufs=1))
        dramp = ctx.enter_context(tc.tile_pool(name="dramp", bufs=1, space="DRAM"))

        # ---- load constants / weights / activations ----
        xcf = singles.tile([128, 2, 30 * 58], BF16)
        nc.sync.dma_start(out=xcf[:, :, :], in_=xcf_d[:, :, :])
        woff = singles.tile([128, 2, KK, 18], BF16)
        nc.sync.dma_start(out=woff[:, :, :, :], in_=woff_d[:, :, :, :])
        boff = singles.tile([18, 1], F32)
        nc.sync.dma_start(out=boff[:, :], in_=boff_d[:, :])
        wm = singles.tile([128, KK, 2, 2, 128], BF16)
        nc.sync.dma_start(out=wm[:, :, :, :, :], in_=wm_d[:, :, :, :, :])

        cf32 = singles.tile([128, NBLK + 1 + 2 * KK], F32)
        nc.sync.dma_start(out=cf32[:, :], in_=cf32_d[:, :])
        ident_f = singles.tile([128, 128], F32)
        nc.scalar.dma_start(out=ident_f[:, :], in_=identf_d[:, :])
        ident_b = singles.tile([128, 128], BF16)
        nc.scalar.dma_start(out=ident_b[:, :], in_=identb_d[:, :])
        iy2 = cf32[:, 0:NBLK]
        jx = cf32[:, NBLK : NBLK + 1]
        kyM = cf32[:, NBLK + 1 : NBLK + 1 + KK]
        kxM = cf32[:, NBLK + 1 + KK : NBLK + 1 + 2 * KK]

        # warmup gather: the first DMAGatherAnt on a core pays a ~12us
        # one-time cost (Q7 overlay load); hide it under the offset conv
        widx = singles.tile([128, 8], I16)
        nc.vector.memset(widx[:, :], 0)
        wg = singles.tile([128, 1, 1024], BF16)
        nc.gpsimd.dma_gather(
            out_ap=wg[:, :, :],
            in_ap=xq_d[:, :],
            idxs_ap=widx[:, :],
            num_idxs=128,
            num_idxs_reg=128,
            elem_size=1024,
            single_packet=False,
        )

        # ---- offset conv + coords + index fold, pipelined in two halves ----
        # half h covers blocks 7h..7h+6 (output pixels 784h..784h+784), so
        # the first seven gathers can start while the second half's offset
        # conv and coordinate math still run.
        off_sb = coords.tile([18, 4 * 392], F32)
        offT = coords.tile([128, NBLK, 18], F32)
        nc.vector.memset(offT[:, :, :], 0.0)
        pym = coords.tile([128, NBLK, KK], F32)
        pxm = coords.tile([128, NBLK, KK], F32)
        ty = coords.tile([128, NBLK, KK], F32)
        tx = coords.tile([128, NBLK, KK], F32)
        tyc = coords.tile([128, NBLK, KK], F32)  # 1 - ty
        txc = coords.tile([128, NBLK, KK], F32)  # 1 - tx
        idxf = coords.tile([128, NBLK, KK], F32)
        idxd = dramp.tile([126, 128], I16)
        idxw = coords.tile([128, NBLK, 72], I16)
        iw = idxw[:, :, :]
        ppw = iw.ap[0][0]
        idb = idxd[:, :]
        HB = 7  # blocks per half
        HC = HB * KK  # 63 idx rows per half

        with (
            tc.tile_pool(name="po", bufs=2, space="PSUM") as po,
            tc.tile_pool(name="pot", bufs=2, space="PSUM") as pot,
            tc.tile_pool(name="pidx", bufs=2, space="PSUM") as pidx,
            tc.tile_pool(name="chalf", bufs=2) as chalf,
        ):
            for h in range(2):
                b0 = h * HB
                for ns in (2 * h, 2 * h + 1):
                    ps_o = po.tile([18, 392], F32)
                    for kc in range(18):
                        k, ch = divmod(kc, 2)
                        ky, kx = divmod(k, 3)
                        rhs = _ap(
                            xcf[:, :, :],
                            ch * 1740 + (ns * 7 + ky) * 58 + kx,
                            [[58, 7], [1, 56]],
                        )
                        nc.tensor.matmul(
                            ps_o[:, :],
                            woff[:, ch, k, :],
                            rhs,
                            start=(kc == 0),
                            stop=(kc == 17),
                        )
                    nc.vector.tensor_scalar(
                        out=off_sb[:, ns * 392 : (ns + 1) * 392],
                        in0=ps_o[:, :],
                        scalar1=boff[:, 0:1],
                        scalar2=None,
                        op0=AL.add,
                    )
                for bb in range(b0, b0 + HB):
                    ps_t = pot.tile([112, 18], F32)
                    nc.tensor.transpose(
                        ps_t[:, :],
                        off_sb[:18, bb * BLK : (bb + 1) * BLK],
                        ident_f[:18, :18],
                    )
                    nc.vector.tensor_copy(out=offT[:112, bb, :], in_=ps_t[:, :])

                # coordinate + weight math on this half's [128, HB, 9] views
                dy = _ap(offT[:], b0 * 18, [[18, HB], [2, KK]])
                dx = _ap(offT[:], b0 * 18 + 1, [[18, HB], [2, KK]])
                iy_b = _ap(iy2, b0, [[1, HB], [0, KK]])
                jx_b = _ap(jx, 0, [[0, HB], [0, KK]])
                kyM_b = _ap(kyM, 0, [[0, HB], [1, KK]])
                kxM_b = _ap(kxM, 0, [[0, HB], [1, KK]])
                pymh = pym[:, b0 : b0 + HB, :]
                pxmh = pxm[:, b0 : b0 + HB, :]
                nc.vector.tensor_tensor(out=pymh, in0=dy, in1=iy_b, op=AL.add)
                nc.vector.tensor_tensor(out=pymh, in0=pymh, in1=kyM_b, op=AL.add)
                nc.vector.tensor_tensor(out=pxmh, in0=dx, in1=jx_b, op=AL.add)
                nc.vector.tensor_tensor(out=pxmh, in0=pxmh, in1=kxM_b, op=AL.add)

                def floor_of(src, nm):
                    ci = chalf.tile([128, HB, KK], I32, tag=f"ci{nm}", name=f"ci{nm}")
                    nc.vector.tensor_copy(out=ci[:, :, :], in_=src)
                    cf = chalf.tile([128, HB, KK], F32, tag=f"cf{nm}", name=f"cf{nm}")
                    nc.vector.tensor_copy(out=cf[:, :, :], in_=ci[:, :, :])
                    gt = chalf.tile([128, HB, KK], F32, tag=f"gt{nm}", name=f"gt{nm}")
                    nc.vector.tensor_tensor(
                        out=gt[:, :, :], in0=cf[:, :, :], in1=src, op=AL.is_gt
                    )
                    nc.vector.tensor_tensor(
                        out=cf[:, :, :], in0=cf[:, :, :], in1=gt[:, :, :],
                        op=AL.subtract,
                    )
                    return cf[:, :, :]

                y0 = floor_of(pymh, "y")
                x0 = floor_of(pxmh, "x")
                tyh = ty[:, b0 : b0 + HB, :]
                txh = tx[:, b0 : b0 + HB, :]
                nc.vector.tensor_tensor(out=tyh, in0=pymh, in1=y0, op=AL.subtract)
                nc.vector.tensor_tensor(out=txh, in0=pxmh, in1=x0, op=AL.subtract)
                nc.vector.tensor_scalar(
                    out=tyc[:, b0 : b0 + HB, :], in0=tyh, scalar1=-1.0, scalar2=1.0,
                    op0=AL.mult, op1=AL.add,
                )
                nc.vector.tensor_scalar(
                    out=txc[:, b0 : b0 + HB, :], in0=txh, scalar1=-1.0, scalar2=1.0,
                    op0=AL.mult, op1=AL.add,
                )
                # clamp into quad table (clamped region is zero-padded ->
                # exact); reuse y0/x0 tiles for the clamped values
                nc.vector.tensor_scalar(
                    out=y0, in0=y0, scalar1=0.0, scalar2=float(HQ - 1),
                    op0=AL.max, op1=AL.min,
                )
                nc.vector.tensor_scalar(
                    out=x0, in0=x0, scalar1=0.0, scalar2=float(WQ - 1),
                    op0=AL.max, op1=AL.min,
                )
                idxfh = idxf[:, b0 : b0 + HB, :]
                nc.vector.scalar_tensor_tensor(
                    out=idxfh, in0=y0, scalar=float(WQ), in1=x0,
                    op0=AL.mult, op1=AL.add,
                )

                # fold into SWDGE wrapped layout via DRAM round trip:
                # idxw[q + 16r, bb, k*8+t] = idx(tap k, pixel 16t+q).
                # idxT16 cols are (q, t)-permuted so the wrap read has 16-byte
                # contiguous runs.
                ps_i = pidx.tile([HC, 128], F32, tag="psi", name="psi")
                nc.tensor.transpose(
                    ps_i[:, :], _ap(idxf[:, :, :], b0 * KK, [[1, HC]]),
                    ident_f[:, :],
                )
                idxT16 = chalf.tile([HC, 128], I16, tag="i16", name="i16")
                nc.vector.tensor_copy(
                    out=idxT16[:, :], in_=_ap(ps_i[:, :], 0, [[1, 16], [16, 8]])
                )
                dstd = bass.AP(
                    tensor=idb.tensor,
                    offset=idb.offset + h * HC * 128,
                    ap=[[128, HC], [1, 128]],
                )
                nc.sync.dma_start(out=dstd, in_=idxT16[:, :])
                # wrap read into partitions 0..15 (split over both HWDGE
                # rings; Pool stays gather-only to avoid Q7 library swaps)
                for part, eng in ((0, nc.sync), (1, nc.scalar)):
                    nb = 4 if part == 0 else 3
                    pb0 = h * HB + part * 4
                    dsth = bass.AP(
                        tensor=iw.tensor,
                        offset=iw.offset + pb0 * 72,
                        ap=[[ppw, 16], [72, nb], [8, KK], [1, 8]],
                    )
                    srch = bass.AP(
                        tensor=idb.tensor,
                        offset=idb.offset + pb0 * 128 * KK,
                        ap=[[8, 16], [128 * KK, nb], [128, KK], [1, 8]],
                    )
                    eng.dma_start(out=dsth, in_=srch)
                # replicate to partition groups 1..7 (SBUF->SBUF, 1KB runs)
                rep = HB * 72
                for r in range(1, 8):
                    src = bass.AP(
                        tensor=iw.tensor,
                        offset=iw.offset + b0 * 72,
                        ap=[[ppw, 16], [1, rep]],
                    )
                    dst = bass.AP(
                        tensor=iw.tensor,
                        offset=iw.offset + 16 * r * ppw + b0 * 72,
                        ap=[[ppw, 16], [1, rep]],
                    )
                    eng = nc.sync if r % 2 else nc.scalar
                    eng.dma_start(out=dst, in_=src)

        # ---- gather + lerp + transpose per block; main matmul per group ----
        rhs_buf = singles.tile([128, KK, 2, NSLOT], BF16)
        out_sb = singles.tile([128, 2, NSLOT], F32)
        with (
            tc.tile_pool(name="gp", bufs=3) as gp,
            tc.tile_pool(name="pp", bufs=3) as pp,
            tc.tile_pool(name="ptb", bufs=2, space="PSUM") as ptb,
            tc.tile_pool(name="pm", bufs=2, space="PSUM") as pm,
        ):
            NDVE = 0  # taps whose y-lerp stays on DVE (ACT waits are elastic;
            # moving real work back to DVE measured slower)
            for bb in range(NBLK):
                g = gp.tile([128, KK, 1024], BF16)
                # two sub-gathers per block (taps 0-4, taps 5-8): smaller
                # descriptor batches duck SWDGE ring-space stalls and let the
                # first taps' lerp start earlier
                nc.gpsimd.dma_gather(
                    out_ap=g[:, 0:5, :],
                    in_ap=xq_d[:, :],
                    idxs_ap=idxw[:, bb, 0:40],
                    num_idxs=5 * SLOT,
                    num_idxs_reg=5 * SLOT,
                    elem_size=1024,
                    single_packet=False,
                )
                nc.gpsimd.dma_gather(
                    out_ap=g[:, 5:KK, :],
                    in_ap=xq_d[:, :],
                    idxs_ap=_ap(idxw[:, :, :], bb * 72 + 40, [[1, 32]]),
                    num_idxs=4 * SLOT,
                    num_idxs_reg=4 * SLOT,
                    elem_size=1024,
                    single_packet=False,
                )
                ps_b = ptb.tile([128, KK, 2, 128], BF16)
                for k in range(KK):
                    gk = g[:, k, 0:1024]
                    # quad row layout: [y0x0, y1x0, y0x1, y1x1] * 256ch each.
                    # x-lerp on DVE: TT-sub (2x) + STT mult-add (1x; STT has no
                    # fast uops but 2 ops beat any 3-op formulation on HW).
                    hx = pp.tile([128, 512], BF16, tag="hx", name="hx")
                    nc.vector.tensor_tensor(
                        out=hx[:, :], in0=gk[:, 512:1024], in1=gk[:, 0:512],
                        op=AL.subtract,
                    )
                    nc.vector.scalar_tensor_tensor(
                        out=hx[:, :], in0=hx[:, :], scalar=tx[:, bb, k : k + 1],
                        in1=gk[:, 0:512], op0=AL.mult, op1=AL.add,
                    )
                    # hx = [y0 x-lerped (256) | y1 x-lerped (256)]
                    # y-lerp split between DVE (sub+STT) and the Scalar engine
                    # (two per-pixel scalings + DVE add) to balance both
                    p0 = pp.tile([128, 256], BF16, tag="p0", name="p0")
                    if k < NDVE:
                        nc.vector.tensor_tensor(
                            out=p0[:, :], in0=hx[:, 256:512], in1=hx[:, 0:256],
                            op=AL.subtract,
                        )
                        nc.vector.scalar_tensor_tensor(
                            out=p0[:, :], in0=p0[:, :],
                            scalar=ty[:, bb, k : k + 1], in1=hx[:, 0:256],
                            op0=AL.mult, op1=AL.add,
                        )
                    else:
                        p1 = pp.tile([128, 256], BF16, tag="p1", name="p1")
                        nc.scalar.activation(
                            out=p0[:, :], in_=hx[:, 0:256],
                            func=mybir.ActivationFunctionType.Copy,
                            scale=tyc[:, bb, k : k + 1],
                        )
                        nc.scalar.activation(
                            out=p1[:, :], in_=hx[:, 256:512],
                            func=mybir.ActivationFunctionType.Copy,
                            scale=ty[:, bb, k : k + 1],
                        )
                        nc.vector.tensor_tensor(
                            out=p0[:, :], in0=p0[:, :], in1=p1[:, :], op=AL.add
                        )
                    for ch in range(2):
                        nc.tensor.transpose(
                            ps_b[:, k, ch, :],
                            p0[:, ch * 128 : (ch + 1) * 128],
                            ident_b[:, :],
                        )
                nc.scalar.copy(
                    out=rhs_buf[:, :, :, bb * SLOT : (bb + 1) * SLOT],
                    in_=ps_b[:, :, :, :],
                )

            # main conv matmul per 512-col group; each group's matmuls only
            # depend on its own four blocks' rhs columns, so they overlap
            # the remaining blocks' gathers/lerps
            for c0, wdt in GROUPS:
                for ot in range(2):
                    ps = pm.tile([128, 512], F32, tag="pm", name="pmtile")
                    for kc in range(18):
                        k, ch = divmod(kc, 2)
                        nc.tensor.matmul(
                            ps[:, 0:wdt],
                            wm[:, k, ch, ot, :],
                            rhs_buf[:, k, ch, c0 : c0 + wdt],
                            start=(kc == 0),
                            stop=(kc == 17),
                        )
                    nc.scalar.copy(
                        out=out_sb[:, ot, c0 : c0 + wdt], in_=ps[:, 0:wdt]
                    )
                nc.sync.dma_start(
                    out=out_d[:, :, c0 : c0 + wdt], in_=out_sb[:, :, c0 : c0 + wdt]
                )

    nc.compile()
    return nc


def prep_inputs(x, w_off, b_off, w):
    """Host-side slab/layout prep. Returns list of 8 per-core input dicts."""
    x = np.asarray(x, dtype=np.float32)
    w_off = np.asarray(w_off, dtype=np.float32)
    b_off = np.asarray(b_off, dtype=np.float32)
    w = np.asarray(w, dtype=np.float32)

    woff_arr = np.ascontiguousarray(
        w_off.reshape(18, 2, 128, KK).transpose(2, 1, 3, 0)
    ).astype(ml_dtypes.bfloat16)  # [128 cl, 2 ch, 9 k, 18 o]
    boff_arr = np.ascontiguousarray(b_off.reshape(18, 1))
    wm_arr = np.ascontiguousarray(
        w.reshape(2, 128, 2, 128, KK).transpose(3, 4, 2, 0, 1)
    ).astype(ml_dtypes.bfloat16)  # [128 cl, 9 k, 2 ch, 2 ot, 128 ol]

    # host-precomputed coordinate planes + identities (core-independent)
    p = np.arange(128)
    half_p = (p >= 56).astype(np.float32)
    cf32_arr = np.zeros((128, NBLK + 1 + 2 * KK), np.float32)
    cf32_arr[:, 0:NBLK] = 2 * np.arange(NBLK)[None, :] + half_p[:, None]
    cf32_arr[:, NBLK] = p - 56 * half_p
    k = np.arange(KK)
    cf32_arr[:, NBLK + 1 : NBLK + 1 + KK] = (MARG - 1 + k // 3)[None, :]
    cf32_arr[:, NBLK + 1 + KK :] = (MARG - 1 + k % 3)[None, :]
    identf_arr = np.eye(128, dtype=np.float32)
    identb_arr = np.eye(128).astype(ml_dtypes.bfloat16)

    in_maps = []
    for core in range(8):
        b, half = divmod(core, 2)
        r0 = half * NROWS
        xb = x[b]  # [256, 56, 56]

        xp58 = np.zeros((CIN, 58, 58), np.float32)
        xp58[:, 1:57, 1:57] = xb
        xcf = np.ascontiguousarray(
            xp58[:, r0 : r0 + 30, :].reshape(2, 128, 30 * 58).transpose(1, 0, 2)
        ).astype(ml_dtypes.bfloat16)

        xp = np.zeros((HQ + 1, WQ + 1, CIN), np.float32)
        ylo = max(0, r0 - MARG)
        yhi = min(H, r0 + HQ + 1 - MARG)
        xhwc = xb.transpose(1, 2, 0)
        xp[ylo - (r0 - MARG) : yhi - (r0 - MARG), MARG : MARG + W, :] = xhwc[ylo:yhi]
        # quad row layout [y0x0, y1x0, y0x1, y1x1] so each lerp stage reads
        # a contiguous 512-element half
        quad = np.stack(
            [xp[:-1, :-1], xp[1:, :-1], xp[:-1, 1:], xp[1:, 1:]], axis=2
        )  # [72, 72, 4, 256]
        xq = np.ascontiguousarray(quad.reshape(NQ, 4 * CIN)).astype(ml_dtypes.bfloat16)

        in_maps.append(
            {
                "xcf": xcf,
                "xq": xq,
                "woff": woff_arr,
                "boff": boff_arr,
                "wm": wm_arr,
                "cf32": cf32_arr,
                "identf": identf_arr,
                "identb": identb_arr,
            }
        )
    return in_maps


def unshard_output(results):
    """results: list of 8 per-core out arrays [128, 2, NSLOT] -> [B,COUT,H,W]."""
    out = np.zeros((B, COUT, H, W), np.float32)
    for core in range(8):
        b, half = divmod(core, 2)
        r0 = half * NROWS
        oc = results[core]  # [128 ol, 2 ot, 1792]
        oc = oc.reshape(128, 2, NBLK, SLOT)[:, :, :, :BLK]
        oc = oc.transpose(1, 0, 2, 3).reshape(COUT, NROWS, W)
        out[b, :, r0 : r0 + NROWS, :] = oc
    return out


def kernel(**inputs):
    nc = _CACHE.get("nc")
    if nc is None:
        nc = build_nc()
        _CACHE["nc"] = nc
    in_maps = prep_inputs(
        inputs["x"], inputs["w_off"], inputs["b_off"], inputs["w"]
    )
    res = run_bass_kernel_spmd(nc, in_maps, core_ids=list(range(8)))
    return unshard_output([r["out"] for r in res.results])
